# revision 19
# baseline (speedup 1.0000x reference)
"""Trainium2 Bass kernel for AcousticTextEncoderLayer.

Reference computation (B=16, T=4096, H=512, K=9):
  w = weight_norm(weight_v, weight_g)            # per-out-channel scale
  x_masked = hidden_states * (t < len)           # zero beyond each length
  conv = conv1d(x_masked, w, same pad) + bias    # per-sample temporal conv
  y = where(t < len, conv, hidden_states)        # passthrough beyond length
  y = layernorm(y, gamma, beta); leaky_relu(y, 0.1)

Strategy: the per-(b,t) work splits into "valid" positions (t < len: conv +
LN) and "invalid" positions (t >= len: LN only).  The host packs all valid
positions of all samples into one zero-separated timeline (8 zero columns
between samples so the 9-tap conv never mixes samples), splits it evenly
across the 8 cores, and packs invalid rows into equal per-core blocks.
Every core runs the same program on its slice; the host scatters results
back.  Conv runs as 36 accumulating fp16 matmuls per 128-position tile
(9 taps x 4 input-channel chunks, fp32 PSUM accumulate) with the
weight-norm scale folded into the shipped weights; LN stats via
bn_stats/bn_aggr; normalize + leaky relu fused into one scalar-engine
activation with per-partition scale/bias (Prelu, alpha=0.1).
"""

import math

import numpy as np

B, T, H, K = 16, 4096, 512, 9
SLOPE = 0.1
EPS = 1e-5
NCORES = 8
SEG = 512          # valid-timeline columns per full segment (4 PSUM tiles)
HALO = K // 2      # 4
SEP = HALO         # zero columns between samples (taps reach <= HALO out)
CHUNKS = H // 128  # 4 input-channel chunks

XDT_NP = np.float16   # matmul operand dtype (fp16: same PE rate as bf16, more mantissa)


def _split_sync_waits(nc, mybir, bass_rust, max_w=1):
    """walrus in this env rejects instructions carrying more than one sync
    wait; spread extra waits over preceding same-engine NoOps."""
    for fn in nc.m.functions:
        for bb in fn.blocks:
            out = []
            changed = False
            for inst in bb.instructions:
                si = inst.sync_info
                waits = list(si.on_wait or []) if si is not None else []
                if len(waits) > max_w:
                    extra, keep = waits[:-max_w], waits[-max_w:]
                    while extra:
                        chunk, extra = extra[:max_w], extra[max_w:]
                        nop = mybir.InstNoOp(
                            name=nc.get_next_instruction_name(), ins=[], outs=[]
                        )
                        nop.engine = inst.engine
                        nop.sync_info = bass_rust.SyncInfo(
                            on_wait=chunk, on_update=[]
                        )
                        out.append(nop)
                    inst.sync_info = bass_rust.SyncInfo(
                        on_wait=keep, on_update=list(si.on_update or [])
                    )
                    changed = True
                out.append(inst)
            if changed:
                bb.instructions[:] = out


def _build_program(n_sub, nti, apply_gb, repeat=1):
    import concourse.bass as bass
    import concourse.tile as tile
    import concourse.mybir as mybir
    import bass_rust
    from contextlib import ExitStack

    f32 = mybir.dt.float32
    xdt = mybir.dt.float16

    nc = bass.Bass("TRN2", target_bir_lowering=False, debug=False,
                   num_devices=NCORES)
    Wc = n_sub * 128
    # segment widths: full SEG-wide segments plus one remainder
    seg_widths = [SEG] * (Wc // SEG)
    if Wc % SEG:
        seg_widths.append(Wc % SEG)
    nseg = len(seg_widths)
    xt = nc.dram_tensor("xt", [CHUNKS, 128, Wc + 2 * HALO], xdt,
                        kind="ExternalInput")
    wt = nc.dram_tensor("wt", [K, CHUNKS, 128, H], xdt, kind="ExternalInput")
    cb = nc.dram_tensor("cb", [1, H], f32, kind="ExternalInput")
    yv = nc.dram_tensor("yv", [Wc, H], f32, kind="ExternalOutput")
    xi = yi = None
    if nti:
        xi = nc.dram_tensor("xi", [nti, 128, H], f32, kind="ExternalInput")
        yi = nc.dram_tensor("yi", [nti, 128, H], f32, kind="ExternalOutput")
    gm = bt = None
    if apply_gb:
        gm = nc.dram_tensor("gm", [1, H], f32, kind="ExternalInput")
        bt = nc.dram_tensor("bt", [1, H], f32, kind="ExternalInput")

    AF = mybir.ActivationFunctionType
    OP = mybir.AluOpType

    with tile.TileContext(nc) as tc, ExitStack() as ctx:
        consts = ctx.enter_context(tc.tile_pool(name="consts", bufs=1))
        xpool = ctx.enter_context(tc.tile_pool(name="xpool", bufs=4))
        ipool = ctx.enter_context(tc.tile_pool(name="ipool", bufs=4))
        psum = ctx.enter_context(tc.tile_pool(name="psum", bufs=8, space="PSUM"))
        ypool = ctx.enter_context(tc.tile_pool(name="ypool", bufs=6))
        opool = ctx.enter_context(tc.tile_pool(name="opool", bufs=6))
        spool = ctx.enter_context(tc.tile_pool(name="spool", bufs=8))

        # First segment's strips go first so PE can start as soon as the
        # first weight tile lands; weights stream in in consumption order.
        strips0 = []
        for c in range(CHUNKS):
            strip = xpool.tile([128, seg_widths[0] + 2 * HALO], xdt,
                               tag=f"strip{c}")
            nc.sync.dma_start(out=strip,
                              in_=xt[c, :, 0: seg_widths[0] + 2 * HALO])
            strips0.append(strip)
        wtiles = [None] * (K * CHUNKS)
        for c in range(CHUNKS):
            for k in range(K):
                wti = consts.tile([128, H], xdt, tag=f"w{k}_{c}")
                nc.sync.dma_start(out=wti, in_=wt[k, c, :, :])
                wtiles[k * CHUNKS + c] = wti
        bias_b = consts.tile([128, H], f32, tag="bias_b")
        nc.sync.dma_start(out=bias_b, in_=cb.ap().to_broadcast((128, H)))
        gm_b = bt_b = None
        if apply_gb:
            gm_b = consts.tile([128, H], f32, tag="gm_b")
            nc.sync.dma_start(out=gm_b, in_=gm.ap().to_broadcast((128, H)))
            bt_b = consts.tile([128, H], f32, tag="bt_b")
            nc.sync.dma_start(out=bt_b, in_=bt.ap().to_broadcast((128, H)))
        eps_t = consts.tile([128, 1], f32, tag="eps")
        nc.vector.memset(eps_t, EPS)

        def ln_lrelu(src, dst):
            # LayerNorm over the free dim + leaky relu, into dst.
            st = spool.tile([128, 6], f32, tag="st")
            nc.vector.bn_stats(out=st, in_=src)
            mv = spool.tile([128, 2], f32, tag="mv")
            nc.vector.bn_aggr(out=mv, in_=st)
            sd = spool.tile([128, 1], f32, tag="sd")
            nc.scalar.activation(out=sd, in_=mv[:, 1:2], func=AF.Sqrt,
                                 bias=eps_t, scale=1.0)
            rstd = spool.tile([128, 1], f32, tag="rstd")
            nc.vector.reciprocal(out=rstd, in_=sd)
            nms = spool.tile([128, 1], f32, tag="nms")
            nc.vector.tensor_scalar(out=nms, in0=mv[:, 0:1], scalar1=rstd,
                                    scalar2=-1.0, op0=OP.mult, op1=OP.mult)
            if not apply_gb:
                nc.scalar.activation(out=dst, in_=src, func=AF.Prelu,
                                     bias=nms, scale=rstd, alpha=SLOPE)
            else:
                tmp = spool.tile([128, H], f32, tag="gbtmp")
                nc.scalar.activation(out=tmp, in_=src, func=AF.Identity,
                                     bias=nms, scale=rstd)
                nc.vector.tensor_mul(out=tmp, in0=tmp, in1=gm_b)
                nc.vector.tensor_add(out=tmp, in0=tmp, in1=bt_b)
                nc.scalar.activation(out=dst, in_=tmp, func=AF.Prelu,
                                     alpha=SLOPE)

        def invalid_tile(it):
            xti = ipool.tile([128, H], f32, tag="xi")
            nc.sync.dma_start(out=xti, in_=xi[it, :, :])
            oi = opool.tile([128, H], f32, tag="oi")
            ln_lrelu(xti, oi)
            nc.sync.dma_start(out=yi[it, :, :], in_=oi)

        # Interleave invalid (LN-only) tiles among valid segments so the
        # vector/scalar engines fill PE-wait gaps.  (repeat>1 re-runs the
        # whole body with identical I/O — used only for differential
        # wall-clock timing, never for the graded kernel.)
        for _rep in range(repeat):
          done_inv = 0
          seg_start = 0
          for s, sw in enumerate(seg_widths):
            if s == 0 and _rep == 0:
                strips = strips0
            else:
                strips = []
                for c in range(CHUNKS):
                    strip = xpool.tile([128, sw + 2 * HALO], xdt,
                                       tag=f"strip{c}")
                    nc.sync.dma_start(
                        out=strip,
                        in_=xt[c, :, seg_start: seg_start + sw + 2 * HALO])
                    strips.append(strip)
            for sub in range(sw // 128):
                ps = psum.tile([128, H], f32, tag="ps")
                first = True
                for c in range(CHUNKS):
                    for k in range(K):
                        nc.tensor.matmul(
                            ps,
                            strips[c][:, sub * 128 + k: sub * 128 + k + 128],
                            wtiles[k * CHUNKS + c],
                            start=first,
                            stop=(c == CHUNKS - 1 and k == K - 1),
                        )
                        first = False
                y = ypool.tile([128, H], f32, tag="y")
                nc.vector.tensor_add(out=y, in0=ps, in1=bias_b)
                o = opool.tile([128, H], f32, tag="o")
                ln_lrelu(y, o)
                row0 = seg_start + sub * 128
                nc.sync.dma_start(out=yv[row0: row0 + 128, :], in_=o)
            seg_start += sw
            inv_target = (s + 1) * nti // nseg
            while done_inv < inv_target:
                invalid_tile(done_inv)
                done_inv += 1
        while done_inv < nti:
            invalid_tile(done_inv)
            done_inv += 1

    _split_sync_waits(nc, mybir, bass_rust)
    return nc


def _pack(hidden_states, input_lengths):
    """Build per-core packed inputs + scatter indices."""
    x = np.ascontiguousarray(np.asarray(hidden_states, dtype=np.float32))
    lens = np.asarray(input_lengths).astype(np.int64).clip(0, T)

    V = int(lens.sum())
    starts = np.zeros(B, np.int64)
    col = 0
    for b in range(B):
        starts[b] = col
        col += int(lens[b]) + SEP
    Wt = col
    n_sub = max(1, math.ceil(math.ceil(Wt / NCORES) / 128))
    Wc = n_sub * 128
    Wtot = NCORES * Wc

    XTL = np.zeros((H, HALO + Wtot + HALO), XDT_NP)
    dest = np.full(Wtot, -1, np.int64)
    for b in range(B):
        L = int(lens[b])
        s0 = int(starts[b])
        XTL[:, HALO + s0: HALO + s0 + L] = x[b, :L, :].T
        dest[s0: s0 + L] = b * T + np.arange(L, dtype=np.int64)

    xts = []
    for m in range(NCORES):
        sl = np.ascontiguousarray(XTL[:, m * Wc: m * Wc + Wc + 2 * HALO])
        xts.append(sl.reshape(CHUNKS, 128, Wc + 2 * HALO))

    # invalid rows
    inv_mask = (np.arange(T)[None, :] >= lens[:, None]).ravel()
    inv_idx = np.nonzero(inv_mask)[0]
    I = len(inv_idx)
    nti = math.ceil(I / (NCORES * 128)) if I else 0
    NI = nti * 128
    xis = None
    inv_pad = None
    if nti:
        x_flat = x.reshape(B * T, H)
        xi_all = np.zeros((NCORES * NI, H), np.float32)
        xi_all[:I] = x_flat[inv_idx]
        inv_pad = np.full(NCORES * NI, -1, np.int64)
        inv_pad[:I] = inv_idx
        xis = [np.ascontiguousarray(
            xi_all[m * NI: (m + 1) * NI].reshape(nti, 128, H))
            for m in range(NCORES)]

    return x, n_sub, Wc, dest, xts, nti, NI, inv_pad, xis


_PROGRAM_CACHE = {}


def _run(inputs, trace=False):
    from concourse.bass_utils import run_bass_kernel_spmd

    x, n_sub, Wc, dest, xts, nti, NI, inv_pad, xis = _pack(
        inputs["hidden_states"], inputs["input_lengths"])

    v = np.asarray(inputs["weight_v"], dtype=np.float32)
    g = np.asarray(inputs["weight_g"], dtype=np.float32)
    norm = np.sqrt((v * v).sum(axis=(1, 2), keepdims=True))
    w_eff = g * v / norm                                  # [H_out, H_in, K]
    wt = np.ascontiguousarray(
        w_eff.transpose(2, 1, 0)).reshape(K, CHUNKS, 128, H).astype(XDT_NP)
    cb = np.asarray(inputs["conv_bias"], np.float32).reshape(1, H)
    gamma = np.asarray(inputs["gamma"], np.float32).reshape(H)
    beta = np.asarray(inputs["beta"], np.float32).reshape(H)
    apply_gb = not (np.allclose(gamma, 1.0) and np.allclose(beta, 0.0))

    cache_key = (n_sub, nti, apply_gb)
    nc = _PROGRAM_CACHE.get(cache_key)
    if nc is None:
        nc = _build_program(n_sub, nti, apply_gb)
        _PROGRAM_CACHE[cache_key] = nc

    in_maps = []
    for m in range(NCORES):
        im = {"xt": xts[m], "wt": wt, "cb": cb}
        if nti:
            im["xi"] = xis[m]
        if apply_gb:
            im["gm"] = gamma.reshape(1, H)
            im["bt"] = beta.reshape(1, H)
        in_maps.append(im)

    res = run_bass_kernel_spmd(nc, in_maps, core_ids=list(range(NCORES)),
                               trace=trace)

    y_flat = np.empty((B * T, H), np.float32)
    for m in range(NCORES):
        yvm = np.asarray(res.results[m]["yv"])
        dm = dest[m * Wc: (m + 1) * Wc]
        sel = dm >= 0
        y_flat[dm[sel]] = yvm[sel]
        if nti:
            yim = np.asarray(res.results[m]["yi"]).reshape(NI, H)
            im_idx = inv_pad[m * NI: (m + 1) * NI]
            sel = im_idx >= 0
            y_flat[im_idx[sel]] = yim[sel]

    return y_flat.reshape(B, T, H), res


def kernel(**inputs):
    out, _ = _run(inputs, trace=False)
    return out


# revision 25
# speedup vs baseline: 1.0180x; 1.0180x over previous
"""Trainium2 Bass kernel for AcousticTextEncoderLayer.

Reference computation (B=16, T=4096, H=512, K=9):
  w = weight_norm(weight_v, weight_g)            # per-out-channel scale
  x_masked = hidden_states * (t < len)           # zero beyond each length
  conv = conv1d(x_masked, w, same pad) + bias    # per-sample temporal conv
  y = where(t < len, conv, hidden_states)        # passthrough beyond length
  y = layernorm(y, gamma, beta); leaky_relu(y, 0.1)

Strategy: the per-(b,t) work splits into "valid" positions (t < len: conv +
LN) and "invalid" positions (t >= len: LN only).  The host packs all valid
positions of all samples into one zero-separated timeline (8 zero columns
between samples so the 9-tap conv never mixes samples), splits it evenly
across the 8 cores, and packs invalid rows into equal per-core blocks.
Every core runs the same program on its slice; the host scatters results
back.  Conv runs as 36 accumulating fp16 matmuls per 128-position tile
(9 taps x 4 input-channel chunks, fp32 PSUM accumulate) with the
weight-norm scale folded into the shipped weights; LN stats via
bn_stats/bn_aggr; normalize + leaky relu fused into one scalar-engine
activation with per-partition scale/bias (Prelu, alpha=0.1).
"""

import math

import numpy as np

B, T, H, K = 16, 4096, 512, 9
SLOPE = 0.1
EPS = 1e-5
NCORES = 8
SEG = 512          # valid-timeline columns per full segment (4 PSUM tiles)
HALO = K // 2      # 4
SEP = HALO         # zero columns between samples (taps reach <= HALO out)
CHUNKS = H // 128  # 4 input-channel chunks

XDT_NP = np.float16   # matmul operand dtype (fp16: same PE rate as bf16, more mantissa)
WARMUP_MMS = 32    # throwaway matmuls that warm the PE clock during load


def _split_sync_waits(nc, mybir, bass_rust, max_w=1):
    """walrus in this env rejects instructions carrying more than one sync
    wait; spread extra waits over preceding same-engine NoOps."""
    for fn in nc.m.functions:
        for bb in fn.blocks:
            out = []
            changed = False
            for inst in bb.instructions:
                si = inst.sync_info
                waits = list(si.on_wait or []) if si is not None else []
                if len(waits) > max_w:
                    extra, keep = waits[:-max_w], waits[-max_w:]
                    while extra:
                        chunk, extra = extra[:max_w], extra[max_w:]
                        nop = mybir.InstNoOp(
                            name=nc.get_next_instruction_name(), ins=[], outs=[]
                        )
                        nop.engine = inst.engine
                        nop.sync_info = bass_rust.SyncInfo(
                            on_wait=chunk, on_update=[]
                        )
                        out.append(nop)
                    inst.sync_info = bass_rust.SyncInfo(
                        on_wait=keep, on_update=list(si.on_update or [])
                    )
                    changed = True
                out.append(inst)
            if changed:
                bb.instructions[:] = out


def _build_program(n_sub, nti, apply_gb, repeat=1):
    import concourse.bass as bass
    import concourse.tile as tile
    import concourse.mybir as mybir
    import bass_rust
    from contextlib import ExitStack

    f32 = mybir.dt.float32
    xdt = mybir.dt.float16

    nc = bass.Bass("TRN2", target_bir_lowering=False, debug=False,
                   num_devices=NCORES)
    Wc = n_sub * 128
    # segment widths: full SEG-wide segments plus one remainder
    seg_widths = [SEG] * (Wc // SEG)
    if Wc % SEG:
        seg_widths.append(Wc % SEG)
    nseg = len(seg_widths)
    xt = nc.dram_tensor("xt", [CHUNKS, 128, Wc + 2 * HALO], xdt,
                        kind="ExternalInput")
    wt = nc.dram_tensor("wt", [K, CHUNKS, 128, H], xdt, kind="ExternalInput")
    cb = nc.dram_tensor("cb", [1, H], f32, kind="ExternalInput")
    yv = nc.dram_tensor("yv", [Wc, H], f32, kind="ExternalOutput")
    xi = yi = None
    if nti:
        xi = nc.dram_tensor("xi", [nti, 128, H], f32, kind="ExternalInput")
        yi = nc.dram_tensor("yi", [nti, 128, H], f32, kind="ExternalOutput")
    gm = bt = None
    if apply_gb:
        gm = nc.dram_tensor("gm", [1, H], f32, kind="ExternalInput")
        bt = nc.dram_tensor("bt", [1, H], f32, kind="ExternalInput")

    AF = mybir.ActivationFunctionType
    OP = mybir.AluOpType

    with tile.TileContext(nc) as tc, ExitStack() as ctx:
        consts = ctx.enter_context(tc.tile_pool(name="consts", bufs=1))
        xpool = ctx.enter_context(tc.tile_pool(name="xpool", bufs=4))
        ipool = ctx.enter_context(tc.tile_pool(name="ipool", bufs=4))
        psum = ctx.enter_context(tc.tile_pool(name="psum", bufs=7, space="PSUM"))
        wpsum = ctx.enter_context(tc.tile_pool(name="wpsum", bufs=1, space="PSUM"))
        ypool = ctx.enter_context(tc.tile_pool(name="ypool", bufs=6))
        opool = ctx.enter_context(tc.tile_pool(name="opool", bufs=6))
        spool = ctx.enter_context(tc.tile_pool(name="spool", bufs=8))

        # First segment's strips go first so PE can start as soon as the
        # first weight tile lands; weights stream in in consumption order.
        strips0 = []
        for c in range(CHUNKS):
            strip = xpool.tile([128, seg_widths[0] + 2 * HALO], xdt,
                               tag=f"strip{c}")
            nc.sync.dma_start(out=strip,
                              in_=xt[c, :, 0: seg_widths[0] + 2 * HALO])
            strips0.append(strip)
        wtiles = [None] * (K * CHUNKS)
        for c in range(CHUNKS):
            for k in range(K):
                wti = consts.tile([128, H], xdt, tag=f"w{k}_{c}")
                nc.sync.dma_start(out=wti, in_=wt[k, c, :, :])
                wtiles[k * CHUNKS + c] = wti
        bias_b = consts.tile([128, H], f32, tag="bias_b")
        nc.sync.dma_start(out=bias_b, in_=cb.ap().to_broadcast((128, H)))
        gm_b = bt_b = None
        if apply_gb:
            gm_b = consts.tile([128, H], f32, tag="gm_b")
            nc.sync.dma_start(out=gm_b, in_=gm.ap().to_broadcast((128, H)))
            bt_b = consts.tile([128, H], f32, tag="bt_b")
            nc.sync.dma_start(out=bt_b, in_=bt.ap().to_broadcast((128, H)))
        eps_t = consts.tile([128, 1], f32, tag="eps")
        nc.vector.memset(eps_t, EPS)

        # Warm up the PE clock (HAM gate: 1.2 -> 2.4 GHz after ~3.4us of
        # sustained activity) with throwaway matmuls on a zeroed tile while
        # the first strips/weights are still in flight.  Results go to a
        # scratch PSUM bank nobody reads.
        if WARMUP_MMS:
            wu_src = consts.tile([128, 128], xdt, tag="wu_src")
            nc.vector.memset(wu_src, 0.0)
            wu_ps = wpsum.tile([128, 128], f32, tag="wups")
            for _ in range(WARMUP_MMS):
                nc.tensor.matmul(wu_ps, wu_src, wu_src,
                                 start=True, stop=True)

        def ln_lrelu(src, dst):
            # LayerNorm over the free dim + leaky relu, into dst.
            st = spool.tile([128, 6], f32, tag="st")
            nc.vector.bn_stats(out=st, in_=src)
            mv = spool.tile([128, 2], f32, tag="mv")
            nc.vector.bn_aggr(out=mv, in_=st)
            sd = spool.tile([128, 1], f32, tag="sd")
            nc.scalar.activation(out=sd, in_=mv[:, 1:2], func=AF.Sqrt,
                                 bias=eps_t, scale=1.0)
            rstd = spool.tile([128, 1], f32, tag="rstd")
            nc.vector.reciprocal(out=rstd, in_=sd)
            nms = spool.tile([128, 1], f32, tag="nms")
            nc.vector.tensor_scalar(out=nms, in0=mv[:, 0:1], scalar1=rstd,
                                    scalar2=-1.0, op0=OP.mult, op1=OP.mult)
            if not apply_gb:
                nc.scalar.activation(out=dst, in_=src, func=AF.Prelu,
                                     bias=nms, scale=rstd, alpha=SLOPE)
            else:
                tmp = spool.tile([128, H], f32, tag="gbtmp")
                nc.scalar.activation(out=tmp, in_=src, func=AF.Identity,
                                     bias=nms, scale=rstd)
                nc.vector.tensor_mul(out=tmp, in0=tmp, in1=gm_b)
                nc.vector.tensor_add(out=tmp, in0=tmp, in1=bt_b)
                nc.scalar.activation(out=dst, in_=tmp, func=AF.Prelu,
                                     alpha=SLOPE)

        def invalid_tile(it):
            xti = ipool.tile([128, H], f32, tag="xi")
            nc.sync.dma_start(out=xti, in_=xi[it, :, :])
            oi = opool.tile([128, H], f32, tag="oi")
            ln_lrelu(xti, oi)
            nc.sync.dma_start(out=yi[it, :, :], in_=oi)

        # Interleave invalid (LN-only) tiles among valid segments so the
        # vector/scalar engines fill PE-wait gaps.  (repeat>1 re-runs the
        # whole body with identical I/O — used only for differential
        # wall-clock timing, never for the graded kernel.)
        for _rep in range(repeat):
          done_inv = 0
          seg_start = 0
          for s, sw in enumerate(seg_widths):
            if s == 0 and _rep == 0:
                strips = strips0
            else:
                strips = []
                for c in range(CHUNKS):
                    strip = xpool.tile([128, sw + 2 * HALO], xdt,
                                       tag=f"strip{c}")
                    nc.sync.dma_start(
                        out=strip,
                        in_=xt[c, :, seg_start: seg_start + sw + 2 * HALO])
                    strips.append(strip)
            for sub in range(sw // 128):
                ps = psum.tile([128, H], f32, tag="ps")
                first = True
                for c in range(CHUNKS):
                    for k in range(K):
                        nc.tensor.matmul(
                            ps,
                            strips[c][:, sub * 128 + k: sub * 128 + k + 128],
                            wtiles[k * CHUNKS + c],
                            start=first,
                            stop=(c == CHUNKS - 1 and k == K - 1),
                        )
                        first = False
                y = ypool.tile([128, H], f32, tag="y")
                nc.vector.tensor_add(out=y, in0=ps, in1=bias_b)
                o = opool.tile([128, H], f32, tag="o")
                ln_lrelu(y, o)
                row0 = seg_start + sub * 128
                nc.sync.dma_start(out=yv[row0: row0 + 128, :], in_=o)
            seg_start += sw
            inv_target = min(nti, (s + 2) * nti // max(1, nseg - 1))
            while done_inv < inv_target:
                invalid_tile(done_inv)
                done_inv += 1
        while done_inv < nti:
            invalid_tile(done_inv)
            done_inv += 1

    _split_sync_waits(nc, mybir, bass_rust)
    return nc


def _pack(hidden_states, input_lengths):
    """Build per-core packed inputs + scatter indices."""
    x = np.ascontiguousarray(np.asarray(hidden_states, dtype=np.float32))
    lens = np.asarray(input_lengths).astype(np.int64).clip(0, T)

    V = int(lens.sum())
    starts = np.zeros(B, np.int64)
    col = 0
    for b in range(B):
        starts[b] = col
        col += int(lens[b]) + SEP
    Wt = col
    n_sub = max(1, math.ceil(math.ceil(Wt / NCORES) / 128))
    Wc = n_sub * 128
    Wtot = NCORES * Wc

    XTL = np.zeros((H, HALO + Wtot + HALO), XDT_NP)
    dest = np.full(Wtot, -1, np.int64)
    for b in range(B):
        L = int(lens[b])
        s0 = int(starts[b])
        XTL[:, HALO + s0: HALO + s0 + L] = x[b, :L, :].T
        dest[s0: s0 + L] = b * T + np.arange(L, dtype=np.int64)

    xts = []
    for m in range(NCORES):
        sl = np.ascontiguousarray(XTL[:, m * Wc: m * Wc + Wc + 2 * HALO])
        xts.append(sl.reshape(CHUNKS, 128, Wc + 2 * HALO))

    # invalid rows
    inv_mask = (np.arange(T)[None, :] >= lens[:, None]).ravel()
    inv_idx = np.nonzero(inv_mask)[0]
    I = len(inv_idx)
    nti = math.ceil(I / (NCORES * 128)) if I else 0
    NI = nti * 128
    xis = None
    inv_pad = None
    if nti:
        x_flat = x.reshape(B * T, H)
        xi_all = np.zeros((NCORES * NI, H), np.float32)
        xi_all[:I] = x_flat[inv_idx]
        inv_pad = np.full(NCORES * NI, -1, np.int64)
        inv_pad[:I] = inv_idx
        xis = [np.ascontiguousarray(
            xi_all[m * NI: (m + 1) * NI].reshape(nti, 128, H))
            for m in range(NCORES)]

    return x, n_sub, Wc, dest, xts, nti, NI, inv_pad, xis


_PROGRAM_CACHE = {}


def _run(inputs, trace=False):
    from concourse.bass_utils import run_bass_kernel_spmd

    x, n_sub, Wc, dest, xts, nti, NI, inv_pad, xis = _pack(
        inputs["hidden_states"], inputs["input_lengths"])

    v = np.asarray(inputs["weight_v"], dtype=np.float32)
    g = np.asarray(inputs["weight_g"], dtype=np.float32)
    norm = np.sqrt((v * v).sum(axis=(1, 2), keepdims=True))
    w_eff = g * v / norm                                  # [H_out, H_in, K]
    wt = np.ascontiguousarray(
        w_eff.transpose(2, 1, 0)).reshape(K, CHUNKS, 128, H).astype(XDT_NP)
    cb = np.asarray(inputs["conv_bias"], np.float32).reshape(1, H)
    gamma = np.asarray(inputs["gamma"], np.float32).reshape(H)
    beta = np.asarray(inputs["beta"], np.float32).reshape(H)
    apply_gb = not (np.allclose(gamma, 1.0) and np.allclose(beta, 0.0))

    cache_key = (n_sub, nti, apply_gb)
    nc = _PROGRAM_CACHE.get(cache_key)
    if nc is None:
        nc = _build_program(n_sub, nti, apply_gb)
        _PROGRAM_CACHE[cache_key] = nc

    in_maps = []
    for m in range(NCORES):
        im = {"xt": xts[m], "wt": wt, "cb": cb}
        if nti:
            im["xi"] = xis[m]
        if apply_gb:
            im["gm"] = gamma.reshape(1, H)
            im["bt"] = beta.reshape(1, H)
        in_maps.append(im)

    res = run_bass_kernel_spmd(nc, in_maps, core_ids=list(range(NCORES)),
                               trace=trace)

    y_flat = np.empty((B * T, H), np.float32)
    for m in range(NCORES):
        yvm = np.asarray(res.results[m]["yv"])
        dm = dest[m * Wc: (m + 1) * Wc]
        sel = dm >= 0
        y_flat[dm[sel]] = yvm[sel]
        if nti:
            yim = np.asarray(res.results[m]["yi"]).reshape(NI, H)
            im_idx = inv_pad[m * NI: (m + 1) * NI]
            sel = im_idx >= 0
            y_flat[im_idx[sel]] = yim[sel]

    return y_flat.reshape(B, T, H), res


def kernel(**inputs):
    out, _ = _run(inputs, trace=False)
    return out
